# revision 1
# baseline (speedup 1.0000x reference)
import os, sys
for _p in ("/opt/trn_rl_repo", "/root/.axon_site",
           "/root/.axon_site/_ro/trn_rl_repo", "/root/.axon_site/_ro/pypackages"):
    if os.path.isdir(_p) and _p not in sys.path:
        sys.path.append(_p)
"""ChebNet (K=3, 3 layers) on 8 TRN2 NeuronCores via Bass/Tile.

Math: with M[d,s] = sum of w_e over edges s->d, deg[s] = sum of w_e out of s,
dis = 1/sqrt(deg) (0 if deg==0), diag = 1[deg>0]-1 (lambda_max=2):
  prop(t)  = -dis o (M @ (dis o t)) + diag o t
Per layer: T0=x, T1=prop(T0), T2=2*prop(T1)-T0, h=sigmoid(T0 W0+T1 W1+T2 W2+b).

Sharding: nodes partitioned across 8 cores (12500 each, padded to 12800 =
100 tiles).  Global gather space is quarter-major: block q (of 4) holds rows
r*3200+(p-3200q) for each rank r -> each 25600-row block is the contiguous
output of one quarter-AllGather and one int16 dma_gather window.

Per prop, per core: dma_gather of 512B rows per edge (4 block streams, dst-tile
sorted), then per 128-edge chunk: S[e,d]=(iota[d]==dstloc[e])*w[e] via one DVE
tensor_scalar, PE matmul S.T @ G accumulated in PSUM per dst tile.
"""
import numpy as np
import concourse.bass as bass
import concourse.bacc as bacc
import concourse.mybir as mybir
import concourse.tile as tile
from concourse.masks import make_identity

F32 = mybir.dt.float32
I16 = mybir.dt.int16
I32 = mybir.dt.int32


# ---------------------------------------------------------------- parameters
class Params:
    def __init__(self, n_nodes, n_cores=8, tiles_per_core=100, feat=128,
                 hidden=128, out_dim=64, c0=24, bf16_gather=False,
                 interleave_cc=True):
        assert tiles_per_core % 4 == 0
        self.N = n_nodes
        self.R = n_cores
        self.TPC = tiles_per_core
        self.CHUNK = tiles_per_core * 128          # padded rows per core
        self.NPC = n_nodes // n_cores              # real rows per core
        assert self.NPC * n_cores == n_nodes
        assert self.NPC <= self.CHUNK
        self.QROWS = self.CHUNK // 4               # rows per quarter per core
        self.BLK = self.R * self.QROWS             # rows per global block
        assert self.BLK <= 32768, "int16 gather window"
        self.F = feat
        self.H = hidden
        self.OUT = out_dim
        self.C0 = c0                               # chunks per gather group
        self.bf16 = bf16_gather
        self.interleave_cc = interleave_cc


# ---------------------------------------------------------------- host prep
def _pack_idx(vals):
    """int16 stream [L] -> [128, L//16]; pos j -> (j%16, j//16), replicated x8."""
    base = vals.reshape(-1, 16).T.astype(np.int16)          # [16, S]
    return np.ascontiguousarray(np.tile(base, (8, 1)))      # [128, S]


def _pack_col(vals):
    """fp32 stream [L] -> [128, L//128]; pos j -> (j%128, j//128)."""
    return np.ascontiguousarray(vals.reshape(-1, 128).T.astype(np.float32))


def prep_host(p: Params, x, edge_index, edge_weight):
    """Returns (sched, per_core_inputs).  sched is shared (compile-time);
    per_core_inputs[r] is the dict of ExternalInput arrays for core r."""
    src = np.asarray(edge_index[0], np.int64)
    dst = np.asarray(edge_index[1], np.int64)
    w = np.asarray(edge_weight, np.float32)

    r_dst = dst // p.NPC
    pd = dst % p.NPC
    t_dst = pd // 128
    dl = (pd % 128).astype(np.float32)
    r_src = src // p.NPC
    ps = src % p.NPC
    q_src = ps // p.QROWS
    idx16 = (r_src * p.QROWS + ps % p.QROWS).astype(np.int64)
    t_src = ps // 128
    sl = (ps % 128).astype(np.float32)

    # ---- main (dst-sharded) streams: per core r, per block b, sorted by t
    # cell counts [R, TPC, 4]
    cell_n = np.zeros((p.R, p.TPC, 4), np.int64)
    np.add.at(cell_n, (r_dst, t_dst, q_src), 1)
    cap = np.ceil(cell_n.max(axis=0) / 128).astype(np.int64)   # [TPC, 4] chunks
    cap[:, 0] = np.maximum(cap[:, 0], 1)                       # psum always defined
    cell_chunk_start = np.zeros((p.TPC, 4), np.int64)
    stream_len = np.zeros(4, np.int64)                         # chunks per block
    for b in range(4):
        cs = np.cumsum(cap[:, b])
        cell_chunk_start[1:, b] = cs[:-1]
        stream_len[b] = cs[-1]

    # ---- deg (src-sharded) schedule
    dcell_n = np.zeros((p.R, p.TPC), np.int64)
    np.add.at(dcell_n, (r_src, t_src), 1)
    dcap = np.ceil(dcell_n.max(axis=0) / 128).astype(np.int64)
    dcap = np.maximum(dcap, 1)
    dcell_start = np.zeros(p.TPC, np.int64)
    dcell_start[1:] = np.cumsum(dcap)[:-1]
    dlen = int(np.cumsum(dcap)[-1])

    degc = np.bincount(src, minlength=p.N)
    sched = dict(cap=cap, cell_chunk_start=cell_chunk_start,
                 stream_len=stream_len, dcap=dcap, dcell_start=dcell_start,
                 dlen=dlen, skip_diag=bool((degc > 0).all()))

    per_core = []
    x = np.asarray(x, np.float32)
    for r in range(p.R):
        ins = {}
        xo = np.zeros((p.CHUNK, p.F), np.float32)
        xo[:p.NPC] = x[r * p.NPC:(r + 1) * p.NPC]
        ins["x_own"] = xo

        m = r_dst == r
        for b in range(4):
            mb = m & (q_src == b)
            L = int(stream_len[b]) * 128
            sidx = np.zeros(L, np.int64)
            sdl = np.zeros(L, np.float32)
            sw = np.zeros(L, np.float32)
            tb = t_dst[mb]
            order = np.argsort(tb, kind="stable")
            tb = tb[order]
            # position within cell
            cnt = cell_n[r, :, b]
            starts_per_t = (cell_chunk_start[:, b] * 128)
            first_of_t = np.zeros(p.TPC, np.int64)
            first_of_t[1:] = np.cumsum(cnt)[:-1]
            within = np.arange(tb.size) - first_of_t[tb]
            pos = starts_per_t[tb] + within
            sidx[pos] = idx16[mb][order]
            sdl[pos] = dl[mb][order]
            sw[pos] = w[mb][order]
            ins[f"idx{b}"] = _pack_idx(sidx)
            ins[f"dl{b}"] = _pack_col(sdl)
            ins[f"w{b}"] = _pack_col(sw)

        md = r_src == r
        L = dlen * 128
        ssl = np.zeros(L, np.float32)
        sww = np.zeros(L, np.float32)
        tb = t_src[md]
        order = np.argsort(tb, kind="stable")
        tb = tb[order]
        cnt = dcell_n[r]
        first_of_t = np.zeros(p.TPC, np.int64)
        first_of_t[1:] = np.cumsum(cnt)[:-1]
        within = np.arange(tb.size) - first_of_t[tb]
        pos = dcell_start[tb] * 128 + within
        ssl[pos] = sl[md][order]
        sww[pos] = w[md][order]
        ins["dsl"] = _pack_col(ssl)
        ins["dw"] = _pack_col(sww)
        per_core.append(ins)
    return sched, per_core


# ---------------------------------------------------------------- program
def build_program(p: Params, sched, weights_meta):
    """weights_meta: dict name->shape for W0..b2 inputs (all cores identical)."""
    nc = bacc.Bacc("TRN2", target_bir_lowering=False, debug=False,
                   num_devices=p.R)
    cap = sched["cap"]
    ccs = sched["cell_chunk_start"]
    slen = sched["stream_len"]
    dcap = sched["dcap"]
    dcs = sched["dcell_start"]
    dlen = sched["dlen"]
    skip_diag = sched.get("skip_diag", False)
    C0 = p.C0
    GT = mybir.dt.bfloat16 if p.bf16 else F32
    RG = [list(range(p.R))]

    # ---- I/O
    x_own = nc.dram_tensor("x_own", [p.CHUNK, p.F], F32, kind="ExternalInput")
    idx_in, dl_in, w_in = [], [], []
    for b in range(4):
        idx_in.append(nc.dram_tensor(f"idx{b}", [128, int(slen[b]) * 8], I16,
                                     kind="ExternalInput"))
        dl_in.append(nc.dram_tensor(f"dl{b}", [128, int(slen[b])], F32,
                                    kind="ExternalInput"))
        w_in.append(nc.dram_tensor(f"w{b}", [128, int(slen[b])], F32,
                                   kind="ExternalInput"))
    dsl_in = nc.dram_tensor("dsl", [128, dlen], F32, kind="ExternalInput")
    dw_in = nc.dram_tensor("dw", [128, dlen], F32, kind="ExternalInput")
    Wt, Bt = [], []
    for l, (fi, fo) in enumerate([(p.F, p.H), (p.H, p.H), (p.H, p.OUT)]):
        Wt.append(nc.dram_tensor(f"W{l}", [3, fi, fo], F32, kind="ExternalInput"))
        Bt.append(nc.dram_tensor(f"b{l}", [fo], F32, kind="ExternalInput"))
    out_t = nc.dram_tensor("out", [p.CHUNK, p.OUT], F32, kind="ExternalOutput")

    with tile.TileContext(nc) as tc:
        with (
            tc.tile_pool(name="dram", bufs=1, space="DRAM") as dpool,
            tc.tile_pool(name="const", bufs=1) as cpool,
            tc.tile_pool(name="meta", bufs=3) as mpool,
            tc.tile_pool(name="gbuf", bufs=2) as gpool,
            tc.tile_pool(name="swork", bufs=4) as spool,
            tc.tile_pool(name="twork", bufs=4) as tpool,
            tc.tile_pool(name="psum", bufs=2, space="PSUM") as ppool,
            tc.tile_pool(name="psum2", bufs=2, space="PSUM") as ppool2,
        ):
            # ---------------- DRAM intermediates
            # gather-source shared buffers: set A (h~ / x~), set B (T1~)
            agA = [[dpool.tile([p.BLK, p.F], GT, addr_space="Shared",
                               name=f"agA{l}_{q}") for q in range(4)]
                   for l in range(3)]
            agB = [[dpool.tile([p.BLK, p.F], GT, addr_space="Shared",
                               name=f"agB{l}_{q}") for q in range(4)]
                   for l in range(3)]
            inA = [dpool.tile([p.QROWS, p.F], GT, name=f"inA{q}") for q in range(4)]
            inB = [dpool.tile([p.QROWS, p.F], GT, name=f"inB{q}") for q in range(4)]
            dis_q_in = [dpool.tile([p.QROWS], F32, name=f"disin{q}") for q in range(4)]
            dis_q_out = [dpool.tile([p.BLK], F32, addr_space="Shared",
                                    name=f"disout{q}") for q in range(4)]
            t1_own = dpool.tile([p.CHUNK, p.F], F32, name="t1own")
            h_own = [dpool.tile([p.CHUNK, p.F], F32, name=f"hown{i}") for i in range(2)]

            # ---------------- constants
            WIDE = max(C0, 16)
            iota_f = cpool.tile([128, WIDE, 128], F32)
            ident = cpool.tile([128, 128], F32)
            nc.gpsimd.iota(iota_f[:, :, :], pattern=[[0, WIDE], [1, 128]],
                           base=0, channel_multiplier=0,
                           allow_small_or_imprecise_dtypes=True)
            make_identity(nc, ident[:, :])
            Wsb, Bsb = [], []
            for l, (fi, fo) in enumerate([(p.F, p.H), (p.H, p.H), (p.H, p.OUT)]):
                wl = []
                for k in range(3):
                    wk = cpool.tile([fi, fo], F32, name=f"Wsb{l}_{k}")
                    nc.sync.dma_start(out=wk[:, :], in_=Wt[l][k, :, :])
                    wl.append(wk)
                Wsb.append(wl)
                bl = cpool.tile([fo, 1], F32, name=f"Bsb{l}")
                nc.sync.dma_start(out=bl[:, :],
                                  in_=Bt[l].rearrange("(g o) -> g o", o=1))
                Bsb.append(bl)

            # ---------------- deg pass -> dis/diag [128, TPC]
            deg_sb = cpool.tile([128, p.TPC], F32)
            CG = 64  # meta column group
            ngroups = (dlen + CG - 1) // CG

            def dmeta(g):
                lo, hi = g * CG, min((g + 1) * CG, dlen)
                slt = mpool.tile([128, CG], F32, tag="dsl")
                wt_ = mpool.tile([128, CG], F32, tag="dw")
                nc.sync.dma_start(out=slt[:, :hi - lo], in_=dsl_in[:, lo:hi])
                nc.sync.dma_start(out=wt_[:, :hi - lo], in_=dw_in[:, lo:hi])
                return slt, wt_

            dmeta_tiles = {}
            dswide = {}
            DWS = 16

            def dswide_get(c):
                sg = c // DWS
                if sg not in dswide:
                    g = (sg * DWS) // CG
                    if g not in dmeta_tiles:
                        dmeta_tiles.clear()
                        dmeta_tiles[g] = dmeta(g)
                    slt, wt_ = dmeta_tiles[g]
                    lo = (sg * DWS) % CG
                    n = min(DWS, dlen - sg * DWS)
                    Sw = spool.tile([128, DWS, 128], F32, tag="Sdegw", bufs=2)
                    nc.vector.tensor_tensor(
                        out=Sw[:, :n, :], in0=iota_f[:, :n, :],
                        in1=slt[:, lo:lo + n].to_broadcast([128, n, 128]),
                        op=mybir.AluOpType.is_equal)
                    nc.vector.tensor_tensor(
                        out=Sw[:, :n, :], in0=Sw[:, :n, :],
                        in1=wt_[:, lo:lo + n].to_broadcast([128, n, 128]),
                        op=mybir.AluOpType.mult)
                    dswide.clear()
                    dswide[sg] = Sw
                return dswide[sg]

            ones_col = cpool.tile([128, 1], F32)
            nc.gpsimd.memset(ones_col[:, :], 1.0)
            for t in range(p.TPC):
                dpsum = ppool.tile([128, 1], F32, tag="prop", space="PSUM")
                nchunks = int(dcap[t])
                for j in range(nchunks):
                    c = int(dcs[t]) + j
                    Sw = dswide_get(c)
                    nc.tensor.matmul(dpsum[:, :], lhsT=Sw[:, c % DWS, :],
                                     rhs=ones_col[:, :],
                                     start=(j == 0), stop=(j == nchunks - 1))
                nc.scalar.copy(out=deg_sb[:, t:t + 1], in_=dpsum[:, :])

            mask_sb = cpool.tile([128, p.TPC], F32)
            dis_sb = cpool.tile([128, p.TPC], F32)
            dis2_sb = cpool.tile([128, p.TPC], F32)   # -dis (for T1) not needed; keep 2*diag
            diag_sb = cpool.tile([128, p.TPC], F32)
            tmp_sb = cpool.tile([128, p.TPC], F32)
            nc.vector.tensor_scalar(out=mask_sb[:, :], in0=deg_sb[:, :],
                                    scalar1=0.0, scalar2=None,
                                    op0=mybir.AluOpType.is_gt)
            nc.vector.tensor_scalar(out=tmp_sb[:, :], in0=deg_sb[:, :],
                                    scalar1=1e-30, scalar2=None,
                                    op0=mybir.AluOpType.max)
            nc.vector.reciprocal(out=tmp_sb[:, :], in_=tmp_sb[:, :])
            nc.scalar.sqrt(out=tmp_sb[:, :], in_=tmp_sb[:, :])
            nc.vector.tensor_tensor(out=dis_sb[:, :], in0=tmp_sb[:, :],
                                    in1=mask_sb[:, :], op=mybir.AluOpType.mult)
            nc.vector.tensor_scalar(out=diag_sb[:, :], in0=mask_sb[:, :],
                                    scalar1=-1.0, scalar2=None,
                                    op0=mybir.AluOpType.add)
            nc.vector.tensor_scalar(out=dis2_sb[:, :], in0=diag_sb[:, :],
                                    scalar1=2.0, scalar2=None,
                                    op0=mybir.AluOpType.mult)  # 2*diag
            ndis2_sb = cpool.tile([128, p.TPC], F32)
            nc.vector.tensor_tensor(out=ndis2_sb[:, :], in0=dis_sb[:, :],
                                    in1=dis_sb[:, :], op=mybir.AluOpType.mult)
            nc.vector.tensor_scalar(out=ndis2_sb[:, :], in0=ndis2_sb[:, :],
                                    scalar1=-1.0, scalar2=None,
                                    op0=mybir.AluOpType.mult)  # -dis^2

            # dis quarters -> AllGather -> dis_full blocks
            TPQ = p.TPC // 4
            for q in range(4):
                nc.scalar.dma_start(
                    out=dis_q_in[q].rearrange("(t p) -> p t", p=128),
                    in_=dis_sb[:, q * TPQ:(q + 1) * TPQ])
            for q in range(4):
                nc.gpsimd.collective_compute(
                    "AllGather", mybir.AluOpType.bypass, replica_groups=RG,
                    ins=[dis_q_in[q][:]], outs=[dis_q_out[q][:]])

            # ---------------- x~ = dis o x -> AllGather into set A
            for t in range(p.TPC):
                q, tq = t // TPQ, t % TPQ
                xt = tpool.tile([128, p.F], F32, tag="xs")
                nc.sync.dma_start(out=xt[:, :],
                                  in_=x_own[t * 128:(t + 1) * 128, :])
                xs = tpool.tile([128, p.F], GT, tag="xss")
                nc.vector.tensor_scalar(out=xs[:, :], in0=xt[:, :],
                                        scalar1=dis_sb[:, t:t + 1], scalar2=None,
                                        op0=mybir.AluOpType.mult)
                nc.scalar.dma_start(out=inA[q][tq * 128:(tq + 1) * 128, :],
                                    in_=xs[:, :])
            for q in range(4):
                nc.gpsimd.collective_compute(
                    "AllGather", mybir.AluOpType.bypass, replica_groups=RG,
                    ins=[inA[q][:]], outs=[agA[0][q][:]])

            # ---------------- propagation machinery
            def run_prop(src_bufs, per_tile_fn, quarter_cb=None):
                """psum_t = M @ gather(src_bufs); per_tile_fn(t, psum)."""
                gather_done = [{} for _ in range(4)]  # group -> (Gtile, meta...)

                def ensure_group(b, g):
                    if g in gather_done[b]:
                        return
                    lo_c = g * C0
                    hi_c = min((g + 1) * C0, int(slen[b]))
                    ncols = hi_c - lo_c
                    G = gpool.tile([128, C0, p.F], GT, tag=f"G{b}", bufs=2)
                    it = mpool.tile([128, C0 * 8], I16, tag=f"gi{b}", bufs=2)
                    dt_ = mpool.tile([128, C0], F32, tag=f"gd{b}", bufs=2)
                    wt_ = mpool.tile([128, C0], F32, tag=f"gw{b}", bufs=2)
                    nc.sync.dma_start(out=it[:, :ncols * 8],
                                      in_=idx_in[b][:, lo_c * 8:hi_c * 8])
                    nc.sync.dma_start(out=dt_[:, :ncols], in_=dl_in[b][:, lo_c:hi_c])
                    nc.sync.dma_start(out=wt_[:, :ncols], in_=w_in[b][:, lo_c:hi_c])
                    nc.gpsimd.dma_gather(
                        out_ap=G[:, :ncols, :], in_ap=src_bufs[b][:],
                        idxs_ap=it[:, :ncols * 8], num_idxs=ncols * 128,
                        num_idxs_reg=ncols * 128, elem_size=p.F, single_packet=False)
                    Sw = spool.tile([128, C0, 128], GT, tag=f"Sw{b}", bufs=1)
                    nc.vector.tensor_tensor(
                        out=Sw[:, :ncols, :], in0=iota_f[:, :ncols, :],
                        in1=dt_[:, :ncols].to_broadcast([128, ncols, 128]),
                        op=mybir.AluOpType.is_equal)
                    nc.vector.tensor_tensor(
                        out=Sw[:, :ncols, :], in0=Sw[:, :ncols, :],
                        in1=wt_[:, :ncols].to_broadcast([128, ncols, 128]),
                        op=mybir.AluOpType.mult)
                    keep = {k: v for k, v in gather_done[b].items() if k >= g - 1}
                    keep[g] = (G, Sw)
                    gather_done[b] = keep

                for t in range(p.TPC):
                    psum = ppool.tile([128, p.F], F32, tag="prop", space="PSUM")
                    total = int(cap[t].sum())
                    done = 0
                    for b in range(4):
                        for j in range(int(cap[t, b])):
                            c = int(ccs[t, b]) + j
                            g, col = c // C0, c % C0
                            ensure_group(b, g)
                            G, Sw = gather_done[b][g]
                            nc.tensor.matmul(psum[:, :], lhsT=Sw[:, col, :],
                                             rhs=G[:, col, :],
                                             start=(done == 0),
                                             stop=(done == total - 1))
                            done += 1
                    per_tile_fn(t, psum)
                    if quarter_cb is not None and (t + 1) % (p.TPC // 4) == 0:
                        quarter_cb(t // (p.TPC // 4))

            def transpose_to(dst_sb, src_sb, gdim=128):
                ps = ppool2.tile([128, 128], F32, tag="trp", space="PSUM")
                nc.tensor.transpose(out=ps[:gdim, :src_sb.shape[0]],
                                    in_=src_sb[:, :gdim],
                                    identity=ident[:src_sb.shape[0], :src_sb.shape[0]])
                nc.scalar.copy(out=dst_sb[:, :], in_=ps[:gdim, :src_sb.shape[0]])

            # ---------------- layers
            for layer in range(3):
                fo = [p.H, p.H, p.OUT][layer]
                t0_src = x_own if layer == 0 else h_own[(layer + 1) % 2]

                # ---- prop1: T1 = -dis o psum + diag o T0 ; write t1_own, inB
                def p1_tile(t, psum, t0_src=t0_src):
                    q, tq = t // TPQ, t % TPQ
                    t1 = tpool.tile([128, p.F], F32, tag="t1a")
                    if skip_diag:
                        nc.vector.tensor_scalar(
                            out=t1[:, :], in0=psum[:, :],
                            scalar1=dis_sb[:, t:t + 1], scalar2=-1.0,
                            op0=mybir.AluOpType.mult, op1=mybir.AluOpType.mult)
                        t1s = tpool.tile([128, p.F], GT, tag="t1sa")
                        nc.vector.tensor_scalar(
                            out=t1s[:, :], in0=psum[:, :],
                            scalar1=ndis2_sb[:, t:t + 1], scalar2=None,
                            op0=mybir.AluOpType.mult)
                    else:
                        t0 = tpool.tile([128, p.F], F32, tag="t0a")
                        nc.sync.dma_start(out=t0[:, :],
                                          in_=t0_src[t * 128:(t + 1) * 128, :])
                        nc.vector.tensor_scalar(
                            out=t1[:, :], in0=psum[:, :],
                            scalar1=dis_sb[:, t:t + 1], scalar2=-1.0,
                            op0=mybir.AluOpType.mult, op1=mybir.AluOpType.mult)
                        tmp = tpool.tile([128, p.F], F32, tag="tmpa")
                        nc.vector.tensor_scalar(
                            out=tmp[:, :], in0=t0[:, :],
                            scalar1=diag_sb[:, t:t + 1], scalar2=None,
                            op0=mybir.AluOpType.mult)
                        nc.vector.tensor_tensor(out=t1[:, :], in0=t1[:, :],
                                                in1=tmp[:, :],
                                                op=mybir.AluOpType.add)
                        t1s = tpool.tile([128, p.F], GT, tag="t1sa")
                        nc.vector.tensor_scalar(out=t1s[:, :], in0=t1[:, :],
                                                scalar1=dis_sb[:, t:t + 1],
                                                scalar2=None,
                                                op0=mybir.AluOpType.mult)
                    nc.scalar.dma_start(out=t1_own[t * 128:(t + 1) * 128, :],
                                        in_=t1[:, :])
                    nc.scalar.dma_start(out=inB[q][tq * 128:(tq + 1) * 128, :],
                                        in_=t1s[:, :])

                def b_cc(q, layer=layer):
                    nc.gpsimd.collective_compute(
                        "AllGather", mybir.AluOpType.bypass, replica_groups=RG,
                        ins=[inB[q][:]], outs=[agB[layer][q][:]])

                if p.interleave_cc:
                    run_prop(agA[layer], p1_tile, quarter_cb=b_cc)
                else:
                    run_prop(agA[layer], p1_tile)
                    for q in range(4):
                        b_cc(q)

                # ---- prop2 + fused layer tail
                def p2_tile(t, psum, layer=layer, fo=fo, t0_src=t0_src):
                    q, tq = t // TPQ, t % TPQ
                    t0 = tpool.tile([128, p.F], F32, tag="t0b")
                    nc.sync.dma_start(out=t0[:, :],
                                      in_=t0_src[t * 128:(t + 1) * 128, :])
                    t1 = tpool.tile([128, p.F], F32, tag="t1b")
                    nc.sync.dma_start(out=t1[:, :],
                                      in_=t1_own[t * 128:(t + 1) * 128, :])
                    # T2 = -2 dis o psum (+ 2 diag o T1) - T0
                    t2 = tpool.tile([128, p.F], F32, tag="t2b")
                    nc.vector.tensor_scalar(
                        out=t2[:, :], in0=psum[:, :],
                        scalar1=dis_sb[:, t:t + 1], scalar2=-2.0,
                        op0=mybir.AluOpType.mult, op1=mybir.AluOpType.mult)
                    if not skip_diag:
                        tmp = tpool.tile([128, p.F], F32, tag="tmpb")
                        nc.vector.tensor_scalar(
                            out=tmp[:, :], in0=t1[:, :],
                            scalar1=dis2_sb[:, t:t + 1], scalar2=None,
                            op0=mybir.AluOpType.mult)
                        nc.vector.tensor_tensor(out=t2[:, :], in0=t2[:, :],
                                                in1=tmp[:, :],
                                                op=mybir.AluOpType.add)
                    nc.vector.tensor_tensor(out=t2[:, :], in0=t2[:, :],
                                            in1=t0[:, :],
                                            op=mybir.AluOpType.subtract)
                    # out = sigmoid(T0 W0 + T1 W1 + T2 W2 + b)
                    wps = ppool2.tile([128, 128], F32, tag="wout", space="PSUM")
                    for k, tk in enumerate((t0, t1, t2)):
                        tkT = tpool.tile([128, 128], F32, tag="tkT")
                        transpose_to(tkT, tk)
                        nc.tensor.matmul(wps[:fo, :], lhsT=Wsb[layer][k][:, :],
                                         rhs=tkT[:, :], start=(k == 0),
                                         stop=(k == 2))
                    hT = tpool.tile([fo, 128], F32, tag="hT")
                    nc.scalar.activation(out=hT[:, :], in_=wps[:fo, :],
                                         func=mybir.ActivationFunctionType.Sigmoid,
                                         bias=Bsb[layer][:, :])
                    hps = ppool2.tile([128, 128], F32, tag="hps", space="PSUM")
                    nc.tensor.transpose(out=hps[:, :fo], in_=hT[:, :],
                                        identity=ident[:fo, :fo])
                    h = tpool.tile([128, fo], F32, tag="hsb")
                    nc.scalar.copy(out=h[:, :], in_=hps[:, :fo])
                    if layer == 2:
                        nc.scalar.dma_start(
                            out=out_t[t * 128:(t + 1) * 128, :], in_=h[:, :])
                    else:
                        nc.scalar.dma_start(
                            out=h_own[layer % 2][t * 128:(t + 1) * 128, :],
                            in_=h[:, :])
                        hs = tpool.tile([128, p.F], GT, tag="hss")
                        nc.vector.tensor_scalar(out=hs[:, :], in0=h[:, :],
                                                scalar1=dis_sb[:, t:t + 1],
                                                scalar2=None,
                                                op0=mybir.AluOpType.mult)
                        nc.scalar.dma_start(
                            out=inA[q][tq * 128:(tq + 1) * 128, :], in_=hs[:, :])

                def a_cc(q, layer=layer):
                    nc.gpsimd.collective_compute(
                        "AllGather", mybir.AluOpType.bypass, replica_groups=RG,
                        ins=[inA[q][:]], outs=[agA[layer + 1][q][:]])

                if layer < 2 and p.interleave_cc:
                    run_prop(agB[layer], p2_tile, quarter_cb=a_cc)
                else:
                    run_prop(agB[layer], p2_tile)
                    if layer < 2:
                        for q in range(4):
                            a_cc(q)

    nc.compile()
    return nc


# ---------------------------------------------------------------- numpy oracle
def numpy_reference(x, edge_index, edge_weight, W0, b0, W1, b1, W2, b2):
    n = x.shape[0]
    src, dst = np.asarray(edge_index[0]), np.asarray(edge_index[1])
    w = np.asarray(edge_weight, np.float64)
    deg = np.zeros(n); np.add.at(deg, src, w)
    dis = np.where(deg > 0, 1.0 / np.sqrt(np.maximum(deg, 1e-30)), 0.0)
    lw = -dis[src] * w * dis[dst]
    diag = np.where(deg > 0, 1.0, 0.0) - 1.0

    import scipy.sparse as sp
    A = sp.csr_matrix((lw, (dst, src)), shape=(n, n))

    def prop(t):
        return A @ t + diag[:, None] * t

    def cheb(t0, W, b):
        out = t0 @ W[0]
        t1 = prop(t0)
        out = out + t1 @ W[1]
        t2 = 2 * prop(t1) - t0
        out = out + t2 @ W[2]
        return out + b

    sig = lambda v: 1.0 / (1.0 + np.exp(-v))
    h = sig(cheb(np.asarray(x, np.float64), np.asarray(W0, np.float64), b0))
    h = sig(cheb(h, np.asarray(W1, np.float64), b1))
    return sig(cheb(h, np.asarray(W2, np.float64), b2))


# ---------------------------------------------------------------- entry point
_N, _E = 100000, 3200000
_CACHE = {}


def kernel(x, edge_index, edge_weight, W0, b0, W1, b1, W2, b2):
    import numpy as np
    p = Params(n_nodes=_N)
    sched, per_core = prep_host(p, x, edge_index, edge_weight)
    key = (tuple(sched["stream_len"]), sched["dlen"], int(sched["cap"].sum()))
    if key in _CACHE:
        nc = _CACHE[key]
    else:
        nc = build_program(p, sched, None)
        _CACHE[key] = nc
    in_maps = []
    for r in range(p.R):
        ins = dict(per_core[r])
        ins.update({"W0": np.asarray(W0, np.float32),
                    "b0": np.asarray(b0, np.float32),
                    "W1": np.asarray(W1, np.float32),
                    "b1": np.asarray(b1, np.float32),
                    "W2": np.asarray(W2, np.float32),
                    "b2": np.asarray(b2, np.float32)})
        in_maps.append(ins)
    from concourse import bass_utils
    res = bass_utils.run_bass_kernel_spmd(nc, in_maps,
                                          core_ids=list(range(p.R)))
    out = np.concatenate([res.results[r]["out"][:p.NPC] for r in range(p.R)], 0)
    return out.astype(np.float32)



# revision 13
# speedup vs baseline: 2.0624x; 2.0624x over previous
import os, sys
for _p in ("/opt/trn_rl_repo", "/root/.axon_site",
           "/root/.axon_site/_ro/trn_rl_repo", "/root/.axon_site/_ro/pypackages"):
    if os.path.isdir(_p) and _p not in sys.path:
        sys.path.append(_p)
"""ChebNet (K=3, 3 layers) on 8 TRN2 NeuronCores via Bass/Tile.

Math: with M[d,s] = sum of w_e over edges s->d, deg[s] = sum of w_e out of s,
dis = 1/sqrt(deg) (0 if deg==0), diag = 1[deg>0]-1 (lambda_max=2):
  prop(t)  = -dis o (M @ (dis o t)) + diag o t
Per layer: T0=x, T1=prop(T0), T2=2*prop(T1)-T0, h=sigmoid(T0 W0+T1 W1+T2 W2+b).

Sharding: nodes partitioned across 8 cores (12500 each, padded to 12800 =
100 tiles).  Global gather space is quarter-major: block q (of 4) holds rows
r*3200+(p-3200q) for each rank r -> each 25600-row block is the contiguous
output of one quarter-AllGather and one int16 dma_gather window.

Per prop, per core: dma_gather of 512B rows per edge (4 block streams, dst-tile
sorted), then per 128-edge chunk: S[e,d]=(iota[d]==dstloc[e])*w[e] via one DVE
tensor_scalar, PE matmul S.T @ G accumulated in PSUM per dst tile.
"""
import numpy as np
import concourse.bass as bass
import concourse.bacc as bacc
import concourse.mybir as mybir
import concourse.tile as tile
from concourse.masks import make_identity

F32 = mybir.dt.float32
I16 = mybir.dt.int16
I32 = mybir.dt.int32


# ---------------------------------------------------------------- parameters
class Params:
    def __init__(self, n_nodes, n_cores=8, tiles_per_core=100, feat=128,
                 hidden=128, out_dim=64, c0=24, bf16_gather=True,
                 interleave_cc=True):
        assert tiles_per_core % 4 == 0
        self.N = n_nodes
        self.R = n_cores
        self.TPC = tiles_per_core
        self.CHUNK = tiles_per_core * 128          # padded rows per core
        self.NPC = n_nodes // n_cores              # real rows per core
        assert self.NPC * n_cores == n_nodes
        assert self.NPC <= self.CHUNK
        self.QROWS = self.CHUNK // 4               # rows per quarter per core
        self.BLK = self.R * self.QROWS             # rows per global block
        assert self.BLK <= 32768, "int16 gather window"
        self.F = feat
        self.H = hidden
        self.OUT = out_dim
        self.C0 = c0                               # chunks per gather group
        self.bf16 = bf16_gather
        self.interleave_cc = interleave_cc


# ---------------------------------------------------------------- host prep
def _pack_idx(vals):
    """int16 stream [L] -> [128, L//16]; pos j -> (j%16, j//16), replicated x8."""
    base = vals.reshape(-1, 16).T.astype(np.int16)          # [16, S]
    return np.ascontiguousarray(np.tile(base, (8, 1)))      # [128, S]


def _np_gt(p):
    import ml_dtypes
    return ml_dtypes.bfloat16 if p.bf16 else np.float32


def _pack_col(vals, dtype=np.float32):
    """stream [L] -> [128, L//128]; pos j -> (j%128, j//128)."""
    return np.ascontiguousarray(vals.reshape(-1, 128).T.astype(dtype))


def prep_host(p: Params, x, edge_index, edge_weight):
    """Returns (sched, per_core_inputs).  sched is shared (compile-time);
    per_core_inputs[r] is the dict of ExternalInput arrays for core r."""
    src = np.asarray(edge_index[0], np.int64)
    dst = np.asarray(edge_index[1], np.int64)
    w = np.asarray(edge_weight, np.float32)

    r_dst = dst // p.NPC
    pd = dst % p.NPC
    t_dst = pd // 128
    dl = (pd % 128).astype(np.float32)
    r_src = src // p.NPC
    ps = src % p.NPC
    q_src = ps // p.QROWS
    idx16 = (r_src * p.QROWS + ps % p.QROWS).astype(np.int64)
    t_src = ps // 128
    sl = (ps % 128).astype(np.float32)

    # ---- main (dst-sharded) streams: per core r, per block b, sorted by t
    # cell counts [R, TPC, 4]
    cell_n = np.zeros((p.R, p.TPC, 4), np.int64)
    np.add.at(cell_n, (r_dst, t_dst, q_src), 1)
    cap = np.ceil(cell_n.max(axis=0) / 128).astype(np.int64)   # [TPC, 4] chunks
    cap[:, 0] = np.maximum(cap[:, 0], 1)                       # psum always defined
    cell_chunk_start = np.zeros((p.TPC, 4), np.int64)
    stream_len = np.zeros(4, np.int64)                         # chunks per block
    for b in range(4):
        cs = np.cumsum(cap[:, b])
        cell_chunk_start[1:, b] = cs[:-1]
        stream_len[b] = cs[-1]

    # ---- deg (src-sharded) schedule
    dcell_n = np.zeros((p.R, p.TPC), np.int64)
    np.add.at(dcell_n, (r_src, t_src), 1)
    dcap = np.ceil(dcell_n.max(axis=0) / 128).astype(np.int64)
    dcap = np.maximum(dcap, 1)
    dcell_start = np.zeros(p.TPC, np.int64)
    dcell_start[1:] = np.cumsum(dcap)[:-1]
    dlen = int(np.cumsum(dcap)[-1])

    degc = np.bincount(src, minlength=p.N)
    sched = dict(cap=cap, cell_chunk_start=cell_chunk_start,
                 stream_len=stream_len, dcap=dcap, dcell_start=dcell_start,
                 dlen=dlen, skip_diag=bool((degc > 0).all()))

    per_core = []
    x = np.asarray(x, np.float32)
    for r in range(p.R):
        ins = {}
        xo = np.zeros((p.CHUNK, p.F), np.float32)
        xo[:p.NPC] = x[r * p.NPC:(r + 1) * p.NPC]
        ins["x_own"] = xo

        m = r_dst == r
        for b in range(4):
            mb = m & (q_src == b)
            L = int(stream_len[b]) * 128
            sidx = np.zeros(L, np.int64)
            sdl = np.zeros(L, np.float32)
            sw = np.zeros(L, np.float32)
            tb = t_dst[mb]
            order = np.argsort(tb, kind="stable")
            tb = tb[order]
            # position within cell
            cnt = cell_n[r, :, b]
            starts_per_t = (cell_chunk_start[:, b] * 128)
            first_of_t = np.zeros(p.TPC, np.int64)
            first_of_t[1:] = np.cumsum(cnt)[:-1]
            within = np.arange(tb.size) - first_of_t[tb]
            pos = starts_per_t[tb] + within
            sidx[pos] = idx16[mb][order]
            sdl[pos] = dl[mb][order]
            sw[pos] = w[mb][order]
            ins[f"idx{b}"] = _pack_idx(sidx)
            ins[f"dl{b}"] = _pack_col(sdl, _np_gt(p))
            ins[f"w{b}"] = _pack_col(sw, _np_gt(p))

        md = r_src == r
        L = dlen * 128
        ssl = np.zeros(L, np.float32)
        sww = np.zeros(L, np.float32)
        tb = t_src[md]
        order = np.argsort(tb, kind="stable")
        tb = tb[order]
        cnt = dcell_n[r]
        first_of_t = np.zeros(p.TPC, np.int64)
        first_of_t[1:] = np.cumsum(cnt)[:-1]
        within = np.arange(tb.size) - first_of_t[tb]
        pos = dcell_start[tb] * 128 + within
        ssl[pos] = sl[md][order]
        sww[pos] = w[md][order]
        ins["dsl"] = _pack_col(ssl, _np_gt(p))
        ins["dw"] = _pack_col(sww, _np_gt(p))
        per_core.append(ins)
    return sched, per_core


# ---------------------------------------------------------------- program
def build_program(p: Params, sched, weights_meta):
    """weights_meta: dict name->shape for W0..b2 inputs (all cores identical)."""
    nc = bacc.Bacc("TRN2", target_bir_lowering=False, debug=False,
                   num_devices=p.R, num_swdge_queues=4)
    cap = sched["cap"]
    ccs = sched["cell_chunk_start"]
    slen = sched["stream_len"]
    dcap = sched["dcap"]
    dcs = sched["dcell_start"]
    dlen = sched["dlen"]
    skip_diag = sched.get("skip_diag", False)
    C0 = p.C0
    GT = mybir.dt.bfloat16 if p.bf16 else F32
    RG = [list(range(p.R))]

    # ---- I/O
    x_own = nc.dram_tensor("x_own", [p.CHUNK, p.F], F32, kind="ExternalInput")
    idx_in, dl_in, w_in = [], [], []
    for b in range(4):
        idx_in.append(nc.dram_tensor(f"idx{b}", [128, int(slen[b]) * 8], I16,
                                     kind="ExternalInput"))
        dl_in.append(nc.dram_tensor(f"dl{b}", [128, int(slen[b])], GT,
                                    kind="ExternalInput"))
        w_in.append(nc.dram_tensor(f"w{b}", [128, int(slen[b])], GT,
                                   kind="ExternalInput"))
    dsl_in = nc.dram_tensor("dsl", [128, dlen], GT, kind="ExternalInput")
    dw_in = nc.dram_tensor("dw", [128, dlen], GT, kind="ExternalInput")
    Wt, Bt = [], []
    for l, (fi, fo) in enumerate([(p.F, p.H), (p.H, p.H), (p.H, p.OUT)]):
        Wt.append(nc.dram_tensor(f"W{l}", [3, fi, fo], F32, kind="ExternalInput"))
        Bt.append(nc.dram_tensor(f"b{l}", [fo], F32, kind="ExternalInput"))
    out_t = nc.dram_tensor("out", [p.CHUNK, p.OUT], F32, kind="ExternalOutput")

    with tile.TileContext(nc) as tc:
        with (
            tc.tile_pool(name="dram", bufs=1, space="DRAM") as dpool,
            tc.tile_pool(name="const", bufs=1) as cpool,
            tc.tile_pool(name="meta", bufs=3) as mpool,
            tc.tile_pool(name="gbuf", bufs=2) as gpool,
            tc.tile_pool(name="swork", bufs=4) as spool,
            tc.tile_pool(name="twork", bufs=4) as tpool,
            tc.tile_pool(name="psum", bufs=2, space="PSUM") as ppool,
            tc.tile_pool(name="psum2", bufs=2, space="PSUM") as ppool2,
        ):
            # ---------------- DRAM intermediates
            # gather-source shared buffers: set A (h~ / x~), set B (T1~)
            agA = [[dpool.tile([p.BLK, p.F], GT, addr_space="Shared",
                               name=f"agA{l}_{q}") for q in range(4)]
                   for l in range(3)]
            agB = [[dpool.tile([p.BLK, p.F], GT, addr_space="Shared",
                               name=f"agB{l}_{q}") for q in range(4)]
                   for l in range(3)]
            inA = [dpool.tile([p.QROWS, p.F], GT, name=f"inA{q}") for q in range(4)]
            inB = [dpool.tile([p.QROWS, p.F], GT, name=f"inB{q}") for q in range(4)]
            dis_q_in = [dpool.tile([p.QROWS], F32, name=f"disin{q}") for q in range(4)]
            dis_q_out = [dpool.tile([p.BLK], F32, addr_space="Shared",
                                    name=f"disout{q}") for q in range(4)]
            t1_own = dpool.tile([p.CHUNK, p.F], F32, name="t1own")
            h_own = [dpool.tile([p.CHUNK, p.F], F32, name=f"hown{i}") for i in range(2)]

            # ---------------- constants
            WIDE = max(C0, 16)
            iota_f = cpool.tile([128, WIDE, 128], GT)
            ident = cpool.tile([128, 128], F32)
            nc.gpsimd.iota(iota_f[:, :, :], pattern=[[0, WIDE], [1, 128]],
                           base=0, channel_multiplier=0,
                           allow_small_or_imprecise_dtypes=True)
            make_identity(nc, ident[:, :])
            Wsb, Bsb = [], []
            for l, (fi, fo) in enumerate([(p.F, p.H), (p.H, p.H), (p.H, p.OUT)]):
                wl = []
                for k in range(3):
                    wk = cpool.tile([fi, fo], F32, name=f"Wsb{l}_{k}")
                    nc.sync.dma_start(out=wk[:, :], in_=Wt[l][k, :, :])
                    wl.append(wk)
                Wsb.append(wl)
                bl = cpool.tile([fo, 1], F32, name=f"Bsb{l}")
                nc.sync.dma_start(out=bl[:, :],
                                  in_=Bt[l].rearrange("(g o) -> g o", o=1))
                Bsb.append(bl)

            # ---------------- deg pass -> dis/diag [128, TPC]
            deg_sb = cpool.tile([128, p.TPC], F32)
            CG = 64  # meta column group
            ngroups = (dlen + CG - 1) // CG

            def dmeta(g):
                lo, hi = g * CG, min((g + 1) * CG, dlen)
                slt = mpool.tile([128, CG], GT, tag="dsl")
                wt_ = mpool.tile([128, CG], GT, tag="dw")
                nc.sync.dma_start(out=slt[:, :hi - lo], in_=dsl_in[:, lo:hi])
                nc.sync.dma_start(out=wt_[:, :hi - lo], in_=dw_in[:, lo:hi])
                return slt, wt_

            dmeta_tiles = {}
            dswide = {}
            DWS = 16

            def dswide_get(c):
                sg = c // DWS
                if sg not in dswide:
                    g = (sg * DWS) // CG
                    if g not in dmeta_tiles:
                        dmeta_tiles.clear()
                        dmeta_tiles[g] = dmeta(g)
                    slt, wt_ = dmeta_tiles[g]
                    lo = (sg * DWS) % CG
                    n = min(DWS, dlen - sg * DWS)
                    Sw = spool.tile([128, DWS, 128], GT, tag="Sdegw", bufs=2)
                    nc.vector.tensor_tensor(
                        out=Sw[:, :n, :], in0=iota_f[:, :n, :],
                        in1=slt[:, lo:lo + n].to_broadcast([128, n, 128]),
                        op=mybir.AluOpType.is_equal)
                    nc.vector.tensor_tensor(
                        out=Sw[:, :n, :], in0=Sw[:, :n, :],
                        in1=wt_[:, lo:lo + n].to_broadcast([128, n, 128]),
                        op=mybir.AluOpType.mult)
                    dswide.clear()
                    dswide[sg] = Sw
                return dswide[sg]

            ones_col = cpool.tile([128, 1], GT)
            nc.gpsimd.memset(ones_col[:, :], 1.0)
            for t in range(p.TPC):
                dpsum = ppool.tile([128, 1], F32, tag="prop", space="PSUM")
                nchunks = int(dcap[t])
                for j in range(nchunks):
                    c = int(dcs[t]) + j
                    Sw = dswide_get(c)
                    nc.tensor.matmul(dpsum[:, :], lhsT=Sw[:, c % DWS, :],
                                     rhs=ones_col[:, :],
                                     start=(j == 0), stop=(j == nchunks - 1))
                nc.scalar.copy(out=deg_sb[:, t:t + 1], in_=dpsum[:, :])

            mask_sb = cpool.tile([128, p.TPC], F32)
            dis_sb = cpool.tile([128, p.TPC], F32)
            dis2_sb = cpool.tile([128, p.TPC], F32)   # -dis (for T1) not needed; keep 2*diag
            diag_sb = cpool.tile([128, p.TPC], F32)
            tmp_sb = cpool.tile([128, p.TPC], F32)
            nc.vector.tensor_scalar(out=mask_sb[:, :], in0=deg_sb[:, :],
                                    scalar1=0.0, scalar2=None,
                                    op0=mybir.AluOpType.is_gt)
            nc.vector.tensor_scalar(out=tmp_sb[:, :], in0=deg_sb[:, :],
                                    scalar1=1e-30, scalar2=None,
                                    op0=mybir.AluOpType.max)
            nc.vector.reciprocal(out=tmp_sb[:, :], in_=tmp_sb[:, :])
            nc.scalar.sqrt(out=tmp_sb[:, :], in_=tmp_sb[:, :])
            nc.vector.tensor_tensor(out=dis_sb[:, :], in0=tmp_sb[:, :],
                                    in1=mask_sb[:, :], op=mybir.AluOpType.mult)
            nc.vector.tensor_scalar(out=diag_sb[:, :], in0=mask_sb[:, :],
                                    scalar1=-1.0, scalar2=None,
                                    op0=mybir.AluOpType.add)
            nc.vector.tensor_scalar(out=dis2_sb[:, :], in0=diag_sb[:, :],
                                    scalar1=2.0, scalar2=None,
                                    op0=mybir.AluOpType.mult)  # 2*diag
            ndis2_sb = cpool.tile([128, p.TPC], F32)
            nc.vector.tensor_tensor(out=ndis2_sb[:, :], in0=dis_sb[:, :],
                                    in1=dis_sb[:, :], op=mybir.AluOpType.mult)
            nc.vector.tensor_scalar(out=ndis2_sb[:, :], in0=ndis2_sb[:, :],
                                    scalar1=-1.0, scalar2=None,
                                    op0=mybir.AluOpType.mult)  # -dis^2

            # dis quarters -> AllGather -> dis_full blocks
            TPQ = p.TPC // 4
            for q in range(4):
                nc.scalar.dma_start(
                    out=dis_q_in[q].rearrange("(t p) -> p t", p=128),
                    in_=dis_sb[:, q * TPQ:(q + 1) * TPQ])
            for q in range(4):
                nc.gpsimd.collective_compute(
                    "AllGather", mybir.AluOpType.bypass, replica_groups=RG,
                    ins=[dis_q_in[q][:]], outs=[dis_q_out[q][:]])

            # ---------------- x~ = dis o x -> AllGather into set A
            for t in range(p.TPC):
                q, tq = t // TPQ, t % TPQ
                xt = tpool.tile([128, p.F], F32, tag="xs")
                nc.sync.dma_start(out=xt[:, :],
                                  in_=x_own[t * 128:(t + 1) * 128, :])
                xs = tpool.tile([128, p.F], GT, tag="xss")
                nc.vector.tensor_scalar(out=xs[:, :], in0=xt[:, :],
                                        scalar1=dis_sb[:, t:t + 1], scalar2=None,
                                        op0=mybir.AluOpType.mult)
                nc.scalar.dma_start(out=inA[q][tq * 128:(tq + 1) * 128, :],
                                    in_=xs[:, :])
            for q in range(4):
                nc.gpsimd.collective_compute(
                    "AllGather", mybir.AluOpType.bypass, replica_groups=RG,
                    ins=[inA[q][:]], outs=[agA[0][q][:]])

            # ---------------- propagation machinery
            gsem = [nc.alloc_semaphore(f"gsem{b}") for b in range(4)]

            def run_prop(src_bufs, per_tile_fn, quarter_cb=None):
                """psum_t = M @ gather(src_bufs); per_tile_fn(t, psum)."""
                gather_done = [{} for _ in range(4)]  # group -> (Gtile, meta...)

                def ensure_group(b, g):
                    if g in gather_done[b]:
                        return
                    lo_c = g * C0
                    hi_c = min((g + 1) * C0, int(slen[b]))
                    ncols = hi_c - lo_c
                    G = gpool.tile([128, C0, p.F], GT, tag=f"G{b}", bufs=2)
                    it = mpool.tile([128, C0 * 8], I16, tag=f"gi{b}", bufs=2)
                    dt_ = mpool.tile([128, C0], GT, tag=f"gd{b}", bufs=2)
                    wt_ = mpool.tile([128, C0], GT, tag=f"gw{b}", bufs=2)
                    nc.sync.dma_start(out=it[:, :ncols * 8],
                                      in_=idx_in[b][:, lo_c * 8:hi_c * 8])
                    nc.sync.dma_start(out=dt_[:, :ncols], in_=dl_in[b][:, lo_c:hi_c])
                    nc.sync.dma_start(out=wt_[:, :ncols], in_=w_in[b][:, lo_c:hi_c])
                    nc.gpsimd.dma_gather(
                        out_ap=G[:, :ncols, :], in_ap=src_bufs[b][:],
                        idxs_ap=it[:, :ncols * 8], num_idxs=ncols * 128,
                        num_idxs_reg=ncols * 128, elem_size=p.F,
                        single_packet=False, queue_num=b)
                    Sw = spool.tile([128, C0, 128], GT, tag=f"Sw{b}", bufs=2)
                    nc.vector.tensor_tensor(
                        out=Sw[:, :ncols, :], in0=iota_f[:, :ncols, :],
                        in1=dt_[:, :ncols].to_broadcast([128, ncols, 128]),
                        op=mybir.AluOpType.is_equal)
                    nc.vector.tensor_tensor(
                        out=Sw[:, :ncols, :], in0=Sw[:, :ncols, :],
                        in1=wt_[:, :ncols].to_broadcast([128, ncols, 128]),
                        op=mybir.AluOpType.mult)
                    keep = {k: v for k, v in gather_done[b].items() if k >= g - 1}
                    keep[g] = (G, Sw)
                    gather_done[b] = keep

                for t in range(p.TPC):
                    psum = ppool.tile([128, p.F], F32, tag="prop", space="PSUM")
                    total = int(cap[t].sum())
                    done = 0
                    for b in range(4):
                        for j in range(int(cap[t, b])):
                            c = int(ccs[t, b]) + j
                            g, col = c // C0, c % C0
                            ensure_group(b, g)
                            G, Sw = gather_done[b][g]
                            nc.tensor.matmul(psum[:, :], lhsT=Sw[:, col, :],
                                             rhs=G[:, col, :],
                                             start=(done == 0),
                                             stop=(done == total - 1))
                            done += 1
                    per_tile_fn(t, psum)
                    if quarter_cb is not None and (t + 1) % (p.TPC // 4) == 0:
                        quarter_cb(t // (p.TPC // 4))

            def transpose_to(dst_sb, src_sb, gdim=128):
                ps = ppool2.tile([128, 128], F32, tag="trp", space="PSUM")
                nc.tensor.transpose(out=ps[:gdim, :src_sb.shape[0]],
                                    in_=src_sb[:, :gdim],
                                    identity=ident[:src_sb.shape[0], :src_sb.shape[0]])
                nc.scalar.copy(out=dst_sb[:, :], in_=ps[:gdim, :src_sb.shape[0]])

            # ---------------- layers
            for layer in range(3):
                fo = [p.H, p.H, p.OUT][layer]
                t0_src = x_own if layer == 0 else h_own[(layer + 1) % 2]

                # ---- prop1: T1 = -dis o psum + diag o T0 ; write t1_own, inB
                def p1_tile(t, psum, t0_src=t0_src):
                    q, tq = t // TPQ, t % TPQ
                    t1 = tpool.tile([128, p.F], F32, tag="t1a")
                    if skip_diag:
                        nc.vector.tensor_scalar(
                            out=t1[:, :], in0=psum[:, :],
                            scalar1=dis_sb[:, t:t + 1], scalar2=-1.0,
                            op0=mybir.AluOpType.mult, op1=mybir.AluOpType.mult)
                        t1s = tpool.tile([128, p.F], GT, tag="t1sa")
                        nc.vector.tensor_scalar(
                            out=t1s[:, :], in0=psum[:, :],
                            scalar1=ndis2_sb[:, t:t + 1], scalar2=None,
                            op0=mybir.AluOpType.mult)
                    else:
                        t0 = tpool.tile([128, p.F], F32, tag="t0a")
                        nc.sync.dma_start(out=t0[:, :],
                                          in_=t0_src[t * 128:(t + 1) * 128, :])
                        nc.vector.tensor_scalar(
                            out=t1[:, :], in0=psum[:, :],
                            scalar1=dis_sb[:, t:t + 1], scalar2=-1.0,
                            op0=mybir.AluOpType.mult, op1=mybir.AluOpType.mult)
                        tmp = tpool.tile([128, p.F], F32, tag="tmpa")
                        nc.vector.tensor_scalar(
                            out=tmp[:, :], in0=t0[:, :],
                            scalar1=diag_sb[:, t:t + 1], scalar2=None,
                            op0=mybir.AluOpType.mult)
                        nc.vector.tensor_tensor(out=t1[:, :], in0=t1[:, :],
                                                in1=tmp[:, :],
                                                op=mybir.AluOpType.add)
                        t1s = tpool.tile([128, p.F], GT, tag="t1sa")
                        nc.vector.tensor_scalar(out=t1s[:, :], in0=t1[:, :],
                                                scalar1=dis_sb[:, t:t + 1],
                                                scalar2=None,
                                                op0=mybir.AluOpType.mult)
                    nc.scalar.dma_start(out=t1_own[t * 128:(t + 1) * 128, :],
                                        in_=t1[:, :])
                    nc.scalar.dma_start(out=inB[q][tq * 128:(tq + 1) * 128, :],
                                        in_=t1s[:, :])

                def b_cc(q, layer=layer):
                    nc.gpsimd.collective_compute(
                        "AllGather", mybir.AluOpType.bypass, replica_groups=RG,
                        ins=[inB[q][:]], outs=[agB[layer][q][:]])

                if p.interleave_cc:
                    run_prop(agA[layer], p1_tile, quarter_cb=b_cc)
                else:
                    run_prop(agA[layer], p1_tile)
                    for q in range(4):
                        b_cc(q)

                # ---- prop2 + fused layer tail
                def p2_tile(t, psum, layer=layer, fo=fo, t0_src=t0_src):
                    q, tq = t // TPQ, t % TPQ
                    t0 = tpool.tile([128, p.F], F32, tag="t0b")
                    nc.sync.dma_start(out=t0[:, :],
                                      in_=t0_src[t * 128:(t + 1) * 128, :])
                    t1 = tpool.tile([128, p.F], F32, tag="t1b")
                    nc.sync.dma_start(out=t1[:, :],
                                      in_=t1_own[t * 128:(t + 1) * 128, :])
                    # T2 = -2 dis o psum (+ 2 diag o T1) - T0
                    t2 = tpool.tile([128, p.F], F32, tag="t2b")
                    nc.vector.tensor_scalar(
                        out=t2[:, :], in0=psum[:, :],
                        scalar1=dis_sb[:, t:t + 1], scalar2=-2.0,
                        op0=mybir.AluOpType.mult, op1=mybir.AluOpType.mult)
                    if not skip_diag:
                        tmp = tpool.tile([128, p.F], F32, tag="tmpb")
                        nc.vector.tensor_scalar(
                            out=tmp[:, :], in0=t1[:, :],
                            scalar1=dis2_sb[:, t:t + 1], scalar2=None,
                            op0=mybir.AluOpType.mult)
                        nc.vector.tensor_tensor(out=t2[:, :], in0=t2[:, :],
                                                in1=tmp[:, :],
                                                op=mybir.AluOpType.add)
                    nc.vector.tensor_tensor(out=t2[:, :], in0=t2[:, :],
                                            in1=t0[:, :],
                                            op=mybir.AluOpType.subtract)
                    # out = sigmoid(T0 W0 + T1 W1 + T2 W2 + b)
                    wps = ppool2.tile([128, 128], F32, tag="wout", space="PSUM")
                    for k, tk in enumerate((t0, t1, t2)):
                        tkT = tpool.tile([128, 128], F32, tag="tkT")
                        transpose_to(tkT, tk)
                        nc.tensor.matmul(wps[:fo, :], lhsT=Wsb[layer][k][:, :],
                                         rhs=tkT[:, :], start=(k == 0),
                                         stop=(k == 2))
                    hT = tpool.tile([fo, 128], F32, tag="hT")
                    nc.scalar.activation(out=hT[:, :], in_=wps[:fo, :],
                                         func=mybir.ActivationFunctionType.Sigmoid,
                                         bias=Bsb[layer][:, :])
                    hps = ppool2.tile([128, 128], F32, tag="hps", space="PSUM")
                    nc.tensor.transpose(out=hps[:, :fo], in_=hT[:, :],
                                        identity=ident[:fo, :fo])
                    h = tpool.tile([128, fo], F32, tag="hsb")
                    nc.scalar.copy(out=h[:, :], in_=hps[:, :fo])
                    if layer == 2:
                        nc.scalar.dma_start(
                            out=out_t[t * 128:(t + 1) * 128, :], in_=h[:, :])
                    else:
                        nc.scalar.dma_start(
                            out=h_own[layer % 2][t * 128:(t + 1) * 128, :],
                            in_=h[:, :])
                        hs = tpool.tile([128, p.F], GT, tag="hss")
                        nc.vector.tensor_scalar(out=hs[:, :], in0=h[:, :],
                                                scalar1=dis_sb[:, t:t + 1],
                                                scalar2=None,
                                                op0=mybir.AluOpType.mult)
                        nc.scalar.dma_start(
                            out=inA[q][tq * 128:(tq + 1) * 128, :], in_=hs[:, :])

                def a_cc(q, layer=layer):
                    nc.gpsimd.collective_compute(
                        "AllGather", mybir.AluOpType.bypass, replica_groups=RG,
                        ins=[inA[q][:]], outs=[agA[layer + 1][q][:]])

                if layer < 2 and p.interleave_cc:
                    run_prop(agB[layer], p2_tile, quarter_cb=a_cc)
                else:
                    run_prop(agB[layer], p2_tile)
                    if layer < 2:
                        for q in range(4):
                            a_cc(q)

    nc.compile()
    return nc


# ---------------------------------------------------------------- numpy oracle
def numpy_reference(x, edge_index, edge_weight, W0, b0, W1, b1, W2, b2):
    n = x.shape[0]
    src, dst = np.asarray(edge_index[0]), np.asarray(edge_index[1])
    w = np.asarray(edge_weight, np.float64)
    deg = np.zeros(n); np.add.at(deg, src, w)
    dis = np.where(deg > 0, 1.0 / np.sqrt(np.maximum(deg, 1e-30)), 0.0)
    lw = -dis[src] * w * dis[dst]
    diag = np.where(deg > 0, 1.0, 0.0) - 1.0

    import scipy.sparse as sp
    A = sp.csr_matrix((lw, (dst, src)), shape=(n, n))

    def prop(t):
        return A @ t + diag[:, None] * t

    def cheb(t0, W, b):
        out = t0 @ W[0]
        t1 = prop(t0)
        out = out + t1 @ W[1]
        t2 = 2 * prop(t1) - t0
        out = out + t2 @ W[2]
        return out + b

    sig = lambda v: 1.0 / (1.0 + np.exp(-v))
    h = sig(cheb(np.asarray(x, np.float64), np.asarray(W0, np.float64), b0))
    h = sig(cheb(h, np.asarray(W1, np.float64), b1))
    return sig(cheb(h, np.asarray(W2, np.float64), b2))


# ---------------------------------------------------------------- entry point
_N, _E = 100000, 3200000
_CACHE = {}


def kernel(x, edge_index, edge_weight, W0, b0, W1, b1, W2, b2):
    import numpy as np
    p = Params(n_nodes=_N)
    sched, per_core = prep_host(p, x, edge_index, edge_weight)
    key = (tuple(sched["stream_len"]), sched["dlen"], int(sched["cap"].sum()))
    if key in _CACHE:
        nc = _CACHE[key]
    else:
        nc = build_program(p, sched, None)
        _CACHE[key] = nc
    in_maps = []
    for r in range(p.R):
        ins = dict(per_core[r])
        ins.update({"W0": np.asarray(W0, np.float32),
                    "b0": np.asarray(b0, np.float32),
                    "W1": np.asarray(W1, np.float32),
                    "b1": np.asarray(b1, np.float32),
                    "W2": np.asarray(W2, np.float32),
                    "b2": np.asarray(b2, np.float32)})
        in_maps.append(ins)
    from concourse import bass_utils
    res = bass_utils.run_bass_kernel_spmd(nc, in_maps,
                                          core_ids=list(range(p.R)))
    out = np.concatenate([res.results[r]["out"][:p.NPC] for r in range(p.R)], 0)
    return out.astype(np.float32)

